# revision 17
# baseline (speedup 1.0000x reference)
"""TRN2 Bass kernel for a fused multi-head attention block (B=2, N=2048,
C=1024, 16 heads, head_dim 64, per-head q/k LayerNorm, out projection).

Sharding: 8 NeuronCores = 2 (batch) x 4 (head groups of 4 heads).
Each core computes qkv for its 4 heads, per-head LN + attention, and a
partial output projection; the host sums the 4 partials per batch
(tensor-parallel unshard) and adds proj bias.

All matmuls run in bf16 (fp32 PSUM accumulation).  x is transposed and
cast to bf16 on the host so the kernel never transposes x on the PE.
LayerNorm rstd uses a bit-trick rsqrt on the vector engine so the only
scalar-engine table functions are {square, copy, exp} - one table set,
no ACT_TABLE_LOAD thrash.  Stage B processes heads in pairs whose k/q
tiles live in opposite 64-partition halves: the S matmuls of the two
heads alternate PE row groups, letting LDWEIGHTS overlap in-flight
matmuls and the two streams run concurrently in the array.  Softmax exp
runs on the scalar engine directly from 3-bank PSUM groups (N<=1536 per
activation) writing bf16 E; the rowsum comes free from a ones-column in
the PV matmul; normalization uses reciprocal_approx_fast.
"""

import sys

sys.path.insert(0, "/opt/trn_rl_repo")

import numpy as np
import ml_dtypes

BF16NP = ml_dtypes.bfloat16

# problem shapes (hardcoded; harness contract)
B, NTOK, C = 2, 2048, 1024
NHEADS, HD = 16, 64
EPS = 1e-6
P = 128
KC = C // P  # 8 k-chunks of the C contraction
TCH = NTOK // P  # 16 token chunks
G = NHEADS // 4  # 4 heads per core
GC = G * HD  # 256 cols per section per core
TQ = 512  # q slab width
NSLAB = NTOK // TQ
SCL = HD**-0.5
KGROUPS = [3, 3, 3, 3, 2, 2]  # k-chunk exp groups (sum=16), <=3 banks each
RSQRT_MAGIC = 0x5F3759DF

PROFILE = False  # set True by test harness to capture NTFF exec time
LAST_RESULTS = None

_CACHE = {}


def _build_nc(has_qkv_bias: bool, ln_affine: bool):
    from contextlib import ExitStack
    from concourse import bacc
    import concourse.tile as tile
    from concourse import mybir
    from concourse.bass import ts
    from concourse.masks import make_identity

    F32 = mybir.dt.float32
    BF16 = mybir.dt.bfloat16
    I32 = mybir.dt.int32
    AX = mybir.AxisListType
    ALU = mybir.AluOpType
    ACTF = mybir.ActivationFunctionType

    from concourse import library_config

    nc = bacc.Bacc("TRN2", target_bir_lowering=False, debug=False)
    # host supplies x pre-transposed/bf16 as [t, p, ko, n]: c = ko*128+p
    xT_d = nc.dram_tensor("xT_shard", [TCH, P, KC, P], BF16, kind="ExternalInput")
    wq_d = nc.dram_tensor("wq_shard", [C, 3 * GC], BF16, kind="ExternalInput")
    wp_d = nc.dram_tensor("wp_shard", [GC, C], BF16, kind="ExternalInput")
    if has_qkv_bias:
        qb_d = nc.dram_tensor("qb_shard", [1, 3 * GC], F32, kind="ExternalInput")
    if ln_affine:
        # rows: [qscale rep4 | kscale rep4], [qbias rep4 | kbias rep4]
        ln_d = nc.dram_tensor("ln_rows", [2, 2 * GC], F32, kind="ExternalInput")
    out0_d = nc.dram_tensor("out_part0", [NTOK, C], BF16, kind="ExternalOutput")
    out1_d = nc.dram_tensor("out_part1", [NTOK, C], BF16, kind="ExternalOutput")

    with tile.TileContext(nc) as tc:
        with ExitStack() as ctx:
            persist = ctx.enter_context(tc.tile_pool(name="persist", bufs=1))
            xT = persist.tile([P, KC, NTOK], BF16, name="xT")
            qkT = persist.tile([P, 4, NTOK], BF16, name="qkT")
            vS = persist.tile([P, TCH, G, HD + 1], BF16, name="vS")
            oT = persist.tile([P, 2, NTOK], BF16, name="oT")
            w_r = persist.tile([P, KC, 3 * GC], BF16, name="w_r")
            wp_r = persist.tile([P, 2, C], BF16, name="wp_r")
            identb = persist.tile([P, P], BF16, name="identb")

            nc.gpsimd.load_library(library_config.attn)
            make_identity(nc, identb[:])

            with tc.tile_pool(name="init", bufs=1) as initp:
                t_ones = initp.tile([P, TCH, G], BF16, name="t_ones")
                nc.vector.memset(t_ones[:], 1.0)
                nc.vector.tensor_copy(vS[:, :, :, HD], t_ones[:])
                nc.sync.dma_start(w_r[:], wq_d.rearrange("(ko p) c -> p ko c", p=P))
                nc.sync.dma_start(wp_r[:], wp_d.rearrange("(ko p) c -> p ko c", p=P))
                if has_qkv_bias:
                    qb1 = initp.tile([1, 3 * GC], F32, name="qb1")
                    nc.sync.dma_start(qb1[:], qb_d[:])
                    brep = persist.tile([P, 3 * GC], F32, name="brep")
                    nc.gpsimd.partition_broadcast(brep[:], qb1[:])
                if ln_affine:
                    ln1 = initp.tile([2, 2 * GC], F32, name="ln1")
                    nc.sync.dma_start(ln1[:], ln_d[:])
                    srep = persist.tile([P, 2 * GC], F32, name="srep")
                    lbrep = persist.tile([P, 2 * GC], F32, name="lbrep")
                    nc.gpsimd.partition_broadcast(srep[:], ln1[0:1, :])
                    nc.gpsimd.partition_broadcast(lbrep[:], ln1[1:2, :])

            # prefetch the whole transposed input; pieces land per t-chunk
            for t in range(TCH):
                nc.sync.dma_start(xT[:, :, ts(t, P)], xT_d[t])

            # p2 (2 banks) is shared by all stages: stage-A transpose
            # ping-pong, stage-B osums, stage-C proj accumulators.
            p2 = ctx.enter_context(tc.tile_pool(name="p2", bufs=2, space="PSUM"))

            # ---- stage A: qkv matmul, per-head LN, v staging, q/k transposes
            # Per t-chunk: qkv MMs -> LN chain (DVE stats + bit-trick rsqrt)
            # -> bf16 qkl -> 4 PE transposes into P2 banks (ping-pong).
            # Transposes for chunk t are emitted after chunk t+1's qkv MMs so
            # the PE never stalls on the LN chain.
            with (
                tc.tile_pool(name="pa", bufs=3, space="PSUM") as pa,
                tc.tile_pool(name="qklp", bufs=3) as qklp,
                tc.tile_pool(name="qkrawp", bufs=3) as qkrawp,
                tc.tile_pool(name="stats", bufs=2) as stp,
            ):
                pend = None
                pend2 = []  # [(qkl, t)] awaiting transposes

                def emit_transposes(item):
                    qkl, t = item
                    # two P2 banks, ping-pong: evac of one bank overlaps the
                    # other bank's transpose (same-bank PE-write/engine-read
                    # is serialized by the scheduler)
                    trbA = p2.tile([P, TQ], F32, tag="acc", name="trpA").bitcast(BF16)
                    trbB = p2.tile([P, TQ], F32, tag="acc", name="trpB").bitcast(BF16)
                    for j2 in range(2):
                        tpA = trbA[:, ts(j2, P)]
                        tpB = trbB[:, ts(j2, P)]
                        nc.tensor.transpose(tpA, qkl[:, ts(j2 * 2, P)], identb[:])
                        nc.tensor.transpose(tpB, qkl[:, ts(j2 * 2 + 1, P)], identb[:])
                        nc.scalar.copy(qkT[:, j2 * 2, ts(t, P)], tpA)
                        nc.vector.tensor_copy(qkT[:, j2 * 2 + 1, ts(t, P)], tpB)

                for t in range(TCH):
                    spt = pa.tile([P, 2, TQ], F32, tag="qk", name="sptA")
                    psA = spt[:, 0, :]  # q|k for 4 heads: 512 cols
                    psB = spt[:, 1, 0:GC]  # v: 256 cols
                    for kc in range(KC):
                        nc.tensor.matmul(
                            psA,
                            xT[:, kc, ts(t, P)],
                            w_r[:, kc, 0 : 2 * GC],
                            start=(kc == 0),
                            stop=(kc == KC - 1),
                        )
                        nc.tensor.matmul(
                            psB,
                            xT[:, kc, ts(t, P)],
                            w_r[:, kc, 2 * GC : 3 * GC],
                            start=(kc == 0),
                            stop=(kc == KC - 1),
                        )
                    while pend2:
                        emit_transposes(pend2.pop(0))
                    if has_qkv_bias:
                        nc.vector.tensor_tensor(psA, psA, brep[:, 0 : 2 * GC], ALU.add)
                        nc.vector.tensor_tensor(
                            psB, psB, brep[:, 2 * GC : 3 * GC], ALU.add
                        )

                    # evacuate q|k to SBUF bf16 (frees the PSUM slot early;
                    # downstream DVE ops run from SBUF)
                    qkraw = qkrawp.tile([P, 2 * GC], BF16, tag="qkraw")
                    nc.scalar.copy(qkraw[:], psA)
                    sq = stp.tile([P, 2 * GC], F32, tag="sq")
                    nc.scalar.square(sq[:], qkraw[:])
                    # v staging (bf16), [tok, head, hd]
                    nc.scalar.copy(
                        vS[:, t, :, 0:HD],
                        psB.rearrange("p (g d) -> p g d", d=HD),
                    )

                    # LN stats; small-op chain batched per pair of t-chunks
                    if t % 2 == 0:
                        sums2 = stp.tile([P, 2, 8], F32, tag="sums2")
                        sumsq2 = stp.tile([P, 2, 8], F32, tag="sumsq2")
                        rstd2 = stp.tile([P, 2, 8], F32, tag="rstd2")
                        nmr2 = stp.tile([P, 2, 8], F32, tag="nmr2")
                        pair = []
                    nc.vector.tensor_reduce(
                        sums2[:, t % 2, :],
                        qkraw[:].rearrange("p (g d) -> p g d", d=HD),
                        axis=AX.X,
                        op=ALU.add,
                    )
                    nc.vector.tensor_reduce(
                        sumsq2[:, t % 2, :],
                        sq[:].rearrange("p (g d) -> p g d", d=HD),
                        axis=AX.X,
                        op=ALU.add,
                    )
                    pair.append((t, qkraw))

                    if t % 2 == 1:
                        # var = sumsq/64 - (sums/64)^2; rstd = rsqrt(var+eps)
                        # via 1/var seed + 2 Newton steps (all [128, 16] DVE)
                        sums = sums2[:]
                        v1 = stp.tile([P, 2, 8], F32, tag="v1")
                        nc.vector.tensor_scalar(
                            v1[:], sumsq2[:], 1.0 / HD, EPS, ALU.mult, ALU.add
                        )
                        mm2 = stp.tile([P, 2, 8], F32, tag="mm2")
                        nc.vector.tensor_tensor(mm2[:], sums, sums, ALU.mult)
                        varep = stp.tile([P, 2, 8], F32, tag="varep")
                        nc.vector.scalar_tensor_tensor(
                            varep[:], in0=mm2[:], scalar=-1.0 / (HD * HD),
                            in1=v1[:], op0=ALU.mult, op1=ALU.add,
                        )
                        rcp = stp.tile([P, 2, 8], F32, tag="rcp")
                        nc.vector.reciprocal(rcp[:], varep[:])
                        vh = stp.tile([P, 2, 8], F32, tag="vh")
                        nc.vector.tensor_scalar_mul(vh[:], varep[:], 0.5)
                        yt = stp.tile([P, 2, 8], F32, tag="yt")
                        nc.vector.tensor_scalar(
                            yt[:], rcp[:], 0.433, 0.433, ALU.mult, ALU.add
                        )
                        y = yt[:]
                        t1 = stp.tile([P, 2, 8], F32, tag="t1")
                        t3 = stp.tile([P, 2, 8], F32, tag="t3")
                        for it in range(2):
                            nc.vector.tensor_tensor(t1[:], y, y, ALU.mult)
                            nc.vector.tensor_tensor(t1[:], t1[:], vh[:], ALU.mult)
                            nc.vector.tensor_scalar(
                                t3[:], t1[:], -1.0, 1.5, ALU.mult, ALU.add
                            )
                            dst = rstd2[:] if it == 1 else y
                            nc.vector.tensor_tensor(dst, y, t3[:], ALU.mult)
                        nc.vector.scalar_tensor_tensor(
                            nmr2[:], in0=sums, scalar=-1.0 / HD, in1=rstd2[:],
                            op0=ALU.mult, op1=ALU.mult,
                        )

                        # apply. q needs only the rstd scale: with k fully
                        # normalized, sum_d k_hat = 0 makes q's mean term
                        # cancel exactly in q_hat @ k_hat^T.
                        for tt, qkr in pair:
                            i2 = tt % 2
                            qkl = qklp.tile([P, 2 * GC], BF16, tag="qkl")
                            qv = qkl[:, 0:GC].rearrange("p (g d) -> p g d", d=HD)
                            kv = qkl[:, GC : 2 * GC].rearrange(
                                "p (g d) -> p g d", d=HD
                            )
                            qr = qkr[:, 0:GC].rearrange("p (g d) -> p g d", d=HD)
                            kr = qkr[:, GC : 2 * GC].rearrange(
                                "p (g d) -> p g d", d=HD
                            )
                            nc.vector.tensor_tensor(
                                qv, qr,
                                rstd2[:, i2, 0:4, None].to_broadcast([P, 4, HD]),
                                ALU.mult,
                            )
                            nc.vector.tensor_tensor(
                                kv, kr,
                                rstd2[:, i2, 4:8, None].to_broadcast([P, 4, HD]),
                                ALU.mult,
                            )
                            nc.vector.tensor_tensor(
                                kv, kv,
                                nmr2[:, i2, 4:8, None].to_broadcast([P, 4, HD]),
                                ALU.add,
                            )
                            if ln_affine:
                                nc.vector.tensor_tensor(
                                    qkl[:], qkl[:], srep[:], ALU.mult
                                )
                                nc.vector.tensor_tensor(
                                    qkl[:], qkl[:], lbrep[:], ALU.add
                                )
                            pend2.append((qkl, tt))
                while pend2:
                    emit_transposes(pend2.pop(0))
                    if has_qkv_bias:
                        nc.vector.tensor_tensor(psA, psA, brep[:, 0 : 2 * GC], ALU.add)
                        nc.vector.tensor_tensor(
                            psB, psB, brep[:, 2 * GC : 3 * GC], ALU.add
                        )

                    # LayerNorm stats over head_dim for the 8 q|k segments:
                    # var = sumsq/64 - (sums/64)^2
                    a3 = psA.rearrange("p (g d) -> p g d", d=HD)
                    sq = stp.tile([P, 2 * GC], F32, tag="sq")
                    nc.scalar.square(sq[:], psA)
                    sums = stp.tile([P, 8], F32, tag="sums")
                    nc.vector.tensor_reduce(sums[:], a3, axis=AX.X, op=ALU.add)
                    sumsq = stp.tile([P, 8], F32, tag="sumsq")
                    nc.vector.tensor_reduce(
                        sumsq[:],
                        sq[:].rearrange("p (g d) -> p g d", d=HD),
                        axis=AX.X,
                        op=ALU.add,
                    )
                    v1 = stp.tile([P, 8], F32, tag="v1")
                    nc.vector.tensor_scalar(
                        v1[:], sumsq[:], 1.0 / HD, EPS, ALU.mult, ALU.add
                    )
                    mm2 = stp.tile([P, 8], F32, tag="mm2")
                    nc.vector.tensor_tensor(mm2[:], sums[:], sums[:], ALU.mult)
                    varep = stp.tile([P, 8], F32, tag="varep")
                    nc.vector.scalar_tensor_tensor(
                        varep[:], in0=mm2[:], scalar=-1.0 / (HD * HD), in1=v1[:],
                        op0=ALU.mult, op1=ALU.add,
                    )
                    # rstd = rsqrt(varep) on DVE only (keeps ACT on one table
                    # set): r = 1/varep, seed y0 = 0.433 + 0.433*r (good on
                    # r in [1/3, 3]; LN var of ~N(0,1) data is ~1), then
                    # 3 Newton steps y <- y*(1.5 - 0.5*varep*y^2).
                    rcp = stp.tile([P, 8], F32, tag="rcp")
                    nc.vector.reciprocal(rcp[:], varep[:])
                    vh = stp.tile([P, 8], F32, tag="vh")
                    nc.vector.tensor_scalar_mul(vh[:], varep[:], 0.5)
                    yt = stp.tile([P, 8], F32, tag="yt")
                    nc.vector.tensor_scalar(
                        yt[:], rcp[:], 0.433, 0.433, ALU.mult, ALU.add
                    )
                    y = yt[:]
                    rstd = stp.tile([P, 8], F32, tag="rstd")
                    t1 = stp.tile([P, 8], F32, tag="t1")
                    t3 = stp.tile([P, 8], F32, tag="t3")
                    for it in range(2):
                        nc.vector.tensor_tensor(t1[:], y, y, ALU.mult)
                        nc.vector.tensor_tensor(t1[:], t1[:], vh[:], ALU.mult)
                        nc.vector.tensor_scalar(
                            t3[:], t1[:], -1.0, 1.5, ALU.mult, ALU.add
                        )
                        dst = rstd[:] if it == 1 else y
                        nc.vector.tensor_tensor(dst, y, t3[:], ALU.mult)
                    nmr = stp.tile([P, 8], F32, tag="nmr")
                    nc.vector.scalar_tensor_tensor(
                        nmr[:], in0=sums[:], scalar=-1.0 / HD, in1=rstd[:],
                        op0=ALU.mult, op1=ALU.mult,
                    )

                    qkl = qklp.tile([P, 2 * GC], BF16, tag="qkl")
                    q3 = qkl[:].rearrange("p (g d) -> p g d", d=HD)
                    nc.vector.tensor_tensor(
                        q3, a3, rstd[:, :, None].to_broadcast([P, 8, HD]), ALU.mult
                    )
                    nc.vector.tensor_tensor(
                        q3, q3, nmr[:, :, None].to_broadcast([P, 8, HD]), ALU.add
                    )
                    if ln_affine:
                        nc.vector.tensor_tensor(qkl[:], qkl[:], srep[:], ALU.mult)
                        nc.vector.tensor_tensor(qkl[:], qkl[:], lbrep[:], ALU.add)

                    # v staging (bf16), [tok, head, hd]
                    nc.scalar.copy(
                        vS[:, t, :, 0:HD],
                        psB.rearrange("p (g d) -> p g d", d=HD),
                    )
                    pend = (qkl, t)
                while pend2:
                    emit_transposes(pend2.pop(0))

            # ---- stage B: attention, heads processed in even/odd pairs ----
            # Head pair (2pr, 2pr+1) occupies partition halves 0:64 / 64:128
            # of qkT[:, pr] (q) and qkT[:, 2+pr] (k).  Interleaving the two
            # heads' S matmuls alternates PE row groups so their LDWEIGHTS
            # overlap in-flight matmuls and the matmuls run concurrently.
            def emit_proj_ts(kc2, outd, s4p, trange):
                # partial projection for one head-pair chunk of GC
                for t in trange:
                    pp0 = p2.tile([P, TQ], F32, tag="acc", name="pp0")
                    pp1 = p2.tile([P, TQ], F32, tag="acc", name="pp1")
                    nc.tensor.matmul(
                        pp0, oT[:, kc2, ts(t, P)], wp_r[:, kc2, 0:TQ],
                        start=True, stop=True,
                    )
                    nc.tensor.matmul(
                        pp1, oT[:, kc2, ts(t, P)], wp_r[:, kc2, TQ : 2 * TQ],
                        start=True, stop=True,
                    )
                    for n2, pp in ((0, pp0), (1, pp1)):
                        ob = s4p.tile([P, TQ], BF16, tag="ob")
                        nc.vector.tensor_copy(ob[:], pp)
                        nc.sync.dma_start(outd[ts(t, P), ts(n2, TQ)], ob[:])

            with (
                tc.tile_pool(name="p1", bufs=2, space="PSUM") as p1,
                tc.tile_pool(name="s3e", bufs=4) as ep,
                tc.tile_pool(name="s3r", bufs=4) as rp,
                tc.tile_pool(name="s4", bufs=3) as s4p,
            ):
                for pr in range(2):
                    for s in range(NSLAB):
                        osA = p2.tile([P, TQ], F32, tag="acc", name="osA")
                        osB = p2.tile([P, TQ], F32, tag="acc", name="osB")
                        prev = None
                        k0 = 0
                        for glen in KGROUPS:
                            sA = p1.tile([P, 3, TQ], F32, tag="spt", name="sBa")
                            sB = p1.tile([P, 3, TQ], F32, tag="spt", name="sBb")
                            for j in range(glen):
                                tk = k0 + j
                                nc.tensor.matmul(
                                    sA[:, j],
                                    qkT[0:HD, 2 + pr, ts(tk, P)],
                                    qkT[0:HD, pr, ts(s, TQ)],
                                    start=True,
                                    stop=True,
                                )
                                nc.tensor.matmul(
                                    sB[:, j],
                                    qkT[HD:P, 2 + pr, ts(tk, P)],
                                    qkT[HD:P, pr, ts(s, TQ)],
                                    start=True,
                                    stop=True,
                                )
                            # PV for the previous exp'd group keeps the PE
                            # busy while ACT runs the current exps.
                            if prev is not None:
                                eA, eB, pk0, pglen = prev
                                for j in range(pglen):
                                    tk = pk0 + j
                                    nc.tensor.matmul(
                                        osA[0 : HD + 1, :],
                                        vS[:, tk, 2 * pr, :],
                                        eA[:, j],
                                        start=(tk == 0),
                                        stop=(tk == TCH - 1),
                                    )
                                    nc.tensor.matmul(
                                        osB[0 : HD + 1, :],
                                        vS[:, tk, 2 * pr + 1, :],
                                        eB[:, j],
                                        start=(tk == 0),
                                        stop=(tk == TCH - 1),
                                    )
                            eA = ep.tile([P, 3, TQ], BF16, tag="et")
                            nc.scalar.activation(
                                eA[:, 0:glen], sA[:, 0:glen], ACTF.Exp, scale=SCL
                            )
                            eB = ep.tile([P, 3, TQ], BF16, tag="et")
                            nc.scalar.activation(
                                eB[:, 0:glen], sB[:, 0:glen], ACTF.Exp, scale=SCL
                            )
                            prev = (eA, eB, k0, glen)
                            k0 += glen
                        eA, eB, pk0, pglen = prev
                        for j in range(pglen):
                            tk = pk0 + j
                            nc.tensor.matmul(
                                osA[0 : HD + 1, :],
                                vS[:, tk, 2 * pr, :],
                                eA[:, j],
                                start=(tk == 0),
                                stop=(tk == TCH - 1),
                            )
                            nc.tensor.matmul(
                                osB[0 : HD + 1, :],
                                vS[:, tk, 2 * pr + 1, :],
                                eB[:, j],
                                start=(tk == 0),
                                stop=(tk == TCH - 1),
                            )
                        # evacuate both osums to SBUF right away so the PSUM
                        # slots free for the next unit (a stalled PE gap here
                        # re-throttles the HAM clock); normalize from SBUF.
                        for h2, osum in ((0, osA), (1, osB)):
                            pb = h2 * HD
                            osb = rp.tile([HD + 1, TQ], F32, tag="osb")
                            nc.vector.tensor_copy(osb[:], osum[0 : HD + 1, :])
                            rec = rp.tile([1, TQ], F32, tag="rec")
                            nc.vector.reciprocal(rec[:], osb[HD : HD + 1, :])
                            bcr = rp.tile([HD, TQ], F32, tag="bcr")
                            nc.gpsimd.partition_broadcast(bcr[:], rec[:])
                            nc.vector.tensor_tensor(
                                oT[pb : pb + HD, pr, ts(s, TQ)],
                                osb[0:HD, :],
                                bcr[:],
                                ALU.mult,
                            )
                    # interleave pr=0's projection pass into pr=1's units (2
                    # token-chunks per unit) so the proj MMs fill PE slack
                    # under the exp backlog; pr=1's pass runs at the end.
                    # after the pr=0 head-pair finishes, its projection pass
                    # keeps the PE dense across the pr transition
                    if s == NSLAB - 1 and pr == 0:
                        emit_proj_ts(0, out0_d, s4p, range(TCH))



                # pr=1's projection pass
                emit_proj_ts(1, out1_d, s4p, range(TCH))

    nc.compile()
    return nc


def _get_nc(has_qkv_bias: bool, ln_affine: bool):
    key = (has_qkv_bias, ln_affine)
    if key not in _CACHE:
        _CACHE[key] = _build_nc(*key)
    return _CACHE[key]


def kernel(**inputs) -> np.ndarray:
    global LAST_RESULTS
    from concourse.bass_utils import run_bass_kernel_spmd

    x = np.asarray(inputs["x"], dtype=np.float32)
    qkv_w = np.asarray(inputs["qkv_w"], dtype=np.float32)
    qkv_b = np.asarray(inputs["qkv_b"], dtype=np.float32)
    qn_scale = np.asarray(inputs["qn_scale"], dtype=np.float32)
    qn_bias = np.asarray(inputs["qn_bias"], dtype=np.float32)
    kn_scale = np.asarray(inputs["kn_scale"], dtype=np.float32)
    kn_bias = np.asarray(inputs["kn_bias"], dtype=np.float32)
    proj_w = np.asarray(inputs["proj_w"], dtype=np.float32)
    proj_b = np.asarray(inputs["proj_b"], dtype=np.float32)

    has_qkv_bias = bool(np.any(qkv_b != 0))
    ln_affine = not (
        np.all(qn_scale == 1)
        and np.all(kn_scale == 1)
        and np.all(qn_bias == 0)
        and np.all(kn_bias == 0)
    )
    nc = _get_nc(has_qkv_bias, ln_affine)

    # per-batch transposed bf16 x, laid out [t, p, ko, n] (c = ko*128 + p)
    xT_b = []
    for b in range(B):
        xt = x[b].T.astype(BF16NP)  # [C, NTOK]
        xt = xt.reshape(KC, P, TCH, P).transpose(2, 1, 0, 3)
        xT_b.append(np.ascontiguousarray(xt))

    in_maps = []
    for c in range(8):
        b, g = divmod(c, 4)
        cs = slice(g * GC, (g + 1) * GC)
        wq = np.concatenate(
            [qkv_w[:, cs], qkv_w[:, C:][:, cs], qkv_w[:, 2 * C :][:, cs]], axis=1
        ).astype(BF16NP)
        m = {
            "xT_shard": xT_b[b],
            "wq_shard": np.ascontiguousarray(wq),
            "wp_shard": np.ascontiguousarray(proj_w[cs, :].astype(BF16NP)),
        }
        if has_qkv_bias:
            m["qb_shard"] = np.concatenate(
                [qkv_b[cs], qkv_b[C:][cs], qkv_b[2 * C :][cs]]
            ).reshape(1, 3 * GC)
        if ln_affine:
            m["ln_rows"] = np.stack(
                [
                    np.concatenate([np.tile(qn_scale, G), np.tile(kn_scale, G)]),
                    np.concatenate([np.tile(qn_bias, G), np.tile(kn_bias, G)]),
                ]
            ).astype(np.float32)
        in_maps.append(m)

    res = run_bass_kernel_spmd(
        nc, in_maps, core_ids=list(range(8)), trace=PROFILE
    )
    LAST_RESULTS = res

    out = np.empty((B, NTOK, C), dtype=np.float32)
    for b in range(B):
        acc = res.results[4 * b]["out_part0"].astype(np.float32)
        acc += res.results[4 * b]["out_part1"].astype(np.float32)
        for g in range(1, 4):
            acc += res.results[4 * b + g]["out_part0"].astype(np.float32)
            acc += res.results[4 * b + g]["out_part1"].astype(np.float32)
        out[b] = acc + proj_b[None, :]
    return out


# revision 18
# speedup vs baseline: 1.0694x; 1.0694x over previous
"""TRN2 Bass kernel for a fused multi-head attention block (B=2, N=2048,
C=1024, 16 heads, head_dim 64, per-head q/k LayerNorm, out projection).

Sharding: 8 NeuronCores = 2 (batch) x 4 (head groups of 4 heads).
Each core computes qkv for its 4 heads, per-head LN + attention, and a
partial output projection; the host sums the 4 partials per batch
(tensor-parallel unshard) and adds proj bias.

All matmuls run in bf16 (fp32 PSUM accumulation).  x is transposed and
cast to bf16 on the host so the kernel never transposes x on the PE.
LayerNorm rstd uses a bit-trick rsqrt on the vector engine so the only
scalar-engine table functions are {square, copy, exp} - one table set,
no ACT_TABLE_LOAD thrash.  Stage B processes heads in pairs whose k/q
tiles live in opposite 64-partition halves: the S matmuls of the two
heads alternate PE row groups, letting LDWEIGHTS overlap in-flight
matmuls and the two streams run concurrently in the array.  Softmax exp
runs on the scalar engine directly from 3-bank PSUM groups (N<=1536 per
activation) writing bf16 E; the rowsum comes free from a ones-column in
the PV matmul; normalization uses reciprocal_approx_fast.
"""

import sys

sys.path.insert(0, "/opt/trn_rl_repo")

import numpy as np
import ml_dtypes

BF16NP = ml_dtypes.bfloat16

# problem shapes (hardcoded; harness contract)
B, NTOK, C = 2, 2048, 1024
NHEADS, HD = 16, 64
EPS = 1e-6
P = 128
KC = C // P  # 8 k-chunks of the C contraction
TCH = NTOK // P  # 16 token chunks
G = NHEADS // 4  # 4 heads per core
GC = G * HD  # 256 cols per section per core
TQ = 512  # q slab width
NSLAB = NTOK // TQ
SCL = HD**-0.5
KGROUPS = [3, 3, 3, 3, 2, 2]  # k-chunk exp groups (sum=16), <=3 banks each
RSQRT_MAGIC = 0x5F3759DF

PROFILE = False  # set True by test harness to capture NTFF exec time
LAST_RESULTS = None

_CACHE = {}


def _build_nc(has_qkv_bias: bool, ln_affine: bool):
    from contextlib import ExitStack
    from concourse import bacc
    import concourse.tile as tile
    from concourse import mybir
    from concourse.bass import ts
    from concourse.masks import make_identity

    F32 = mybir.dt.float32
    BF16 = mybir.dt.bfloat16
    I32 = mybir.dt.int32
    AX = mybir.AxisListType
    ALU = mybir.AluOpType
    ACTF = mybir.ActivationFunctionType

    from concourse import library_config

    nc = bacc.Bacc("TRN2", target_bir_lowering=False, debug=False)
    # host supplies x pre-transposed/bf16 as [t, p, ko, n]: c = ko*128+p
    xT_d = nc.dram_tensor("xT_shard", [TCH, P, KC, P], BF16, kind="ExternalInput")
    wq_d = nc.dram_tensor("wq_shard", [C, 3 * GC], BF16, kind="ExternalInput")
    wp_d = nc.dram_tensor("wp_shard", [GC, C], BF16, kind="ExternalInput")
    if has_qkv_bias:
        qb_d = nc.dram_tensor("qb_shard", [1, 3 * GC], F32, kind="ExternalInput")
    if ln_affine:
        # rows: [qscale rep4 | kscale rep4], [qbias rep4 | kbias rep4]
        ln_d = nc.dram_tensor("ln_rows", [2, 2 * GC], F32, kind="ExternalInput")
    out0_d = nc.dram_tensor("out_part0", [NTOK, C], BF16, kind="ExternalOutput")
    out1_d = nc.dram_tensor("out_part1", [NTOK, C], BF16, kind="ExternalOutput")

    with tile.TileContext(nc) as tc:
        with ExitStack() as ctx:
            persist = ctx.enter_context(tc.tile_pool(name="persist", bufs=1))
            xT = persist.tile([P, KC, NTOK], BF16, name="xT")
            qkT = persist.tile([P, 4, NTOK], BF16, name="qkT")
            vS = persist.tile([P, TCH, G, HD + 1], BF16, name="vS")
            oT = persist.tile([P, 2, NTOK], BF16, name="oT")
            w_r = persist.tile([P, KC, 3 * GC], BF16, name="w_r")
            wp_r = persist.tile([P, 2, C], BF16, name="wp_r")
            identb = persist.tile([P, P], BF16, name="identb")

            nc.gpsimd.load_library(library_config.attn)
            make_identity(nc, identb[:])

            with tc.tile_pool(name="init", bufs=1) as initp:
                t_ones = initp.tile([P, TCH, G], BF16, name="t_ones")
                nc.vector.memset(t_ones[:], 1.0)
                nc.vector.tensor_copy(vS[:, :, :, HD], t_ones[:])
                nc.sync.dma_start(w_r[:], wq_d.rearrange("(ko p) c -> p ko c", p=P))
                nc.sync.dma_start(wp_r[:], wp_d.rearrange("(ko p) c -> p ko c", p=P))
                if has_qkv_bias:
                    qb1 = initp.tile([1, 3 * GC], F32, name="qb1")
                    nc.sync.dma_start(qb1[:], qb_d[:])
                    brep = persist.tile([P, 3 * GC], F32, name="brep")
                    nc.gpsimd.partition_broadcast(brep[:], qb1[:])
                if ln_affine:
                    ln1 = initp.tile([2, 2 * GC], F32, name="ln1")
                    nc.sync.dma_start(ln1[:], ln_d[:])
                    srep = persist.tile([P, 2 * GC], F32, name="srep")
                    lbrep = persist.tile([P, 2 * GC], F32, name="lbrep")
                    nc.gpsimd.partition_broadcast(srep[:], ln1[0:1, :])
                    nc.gpsimd.partition_broadcast(lbrep[:], ln1[1:2, :])

            # prefetch the whole transposed input; pieces land per t-chunk
            for t in range(TCH):
                nc.sync.dma_start(xT[:, :, ts(t, P)], xT_d[t])

            # p2 (2 banks) is shared by all stages: stage-A transpose
            # ping-pong, stage-B osums, stage-C proj accumulators.
            p2 = ctx.enter_context(tc.tile_pool(name="p2", bufs=2, space="PSUM"))

            # ---- stage A: qkv matmul, per-head LN, v staging, q/k transposes
            # Per t-chunk: qkv MMs -> LN chain (DVE stats + bit-trick rsqrt)
            # -> bf16 qkl -> 4 PE transposes into P2 banks (ping-pong).
            # Transposes for chunk t are emitted after chunk t+1's qkv MMs so
            # the PE never stalls on the LN chain.
            with (
                tc.tile_pool(name="pa", bufs=3, space="PSUM") as pa,
                tc.tile_pool(name="qklp", bufs=3) as qklp,
                tc.tile_pool(name="qkrawp", bufs=3) as qkrawp,
                tc.tile_pool(name="stats", bufs=2) as stp,
            ):
                pend = None
                pend2 = []  # [(qkl, t)] awaiting transposes

                def emit_transposes(item):
                    # DMA-xbar transposes: Sync queue is idle mid-stage-A and
                    # this keeps the PE stream free of transpose bursts
                    qkl, t = item
                    for j in range(4):
                        nc.sync.dma_start_transpose(
                            qkT[:, j, ts(t, P)], qkl[:, ts(j, P)]
                        )

                for t in range(TCH):
                    spt = pa.tile([P, 2, TQ], F32, tag="qk", name="sptA")
                    psA = spt[:, 0, :]  # q|k for 4 heads: 512 cols
                    psB = spt[:, 1, 0:GC]  # v: 256 cols
                    for kc in range(KC):
                        nc.tensor.matmul(
                            psA,
                            xT[:, kc, ts(t, P)],
                            w_r[:, kc, 0 : 2 * GC],
                            start=(kc == 0),
                            stop=(kc == KC - 1),
                        )
                        nc.tensor.matmul(
                            psB,
                            xT[:, kc, ts(t, P)],
                            w_r[:, kc, 2 * GC : 3 * GC],
                            start=(kc == 0),
                            stop=(kc == KC - 1),
                        )
                    while pend2:
                        emit_transposes(pend2.pop(0))
                    if has_qkv_bias:
                        nc.vector.tensor_tensor(psA, psA, brep[:, 0 : 2 * GC], ALU.add)
                        nc.vector.tensor_tensor(
                            psB, psB, brep[:, 2 * GC : 3 * GC], ALU.add
                        )

                    # evacuate q|k to SBUF bf16 (frees the PSUM slot early;
                    # downstream DVE ops run from SBUF)
                    qkraw = qkrawp.tile([P, 2 * GC], BF16, tag="qkraw")
                    nc.scalar.copy(qkraw[:], psA)
                    sq = stp.tile([P, 2 * GC], F32, tag="sq")
                    nc.scalar.square(sq[:], qkraw[:])
                    # v staging (bf16), [tok, head, hd]
                    nc.scalar.copy(
                        vS[:, t, :, 0:HD],
                        psB.rearrange("p (g d) -> p g d", d=HD),
                    )

                    # LN stats; small-op chain batched per pair of t-chunks
                    if t % 2 == 0:
                        sums2 = stp.tile([P, 2, 8], F32, tag="sums2")
                        sumsq2 = stp.tile([P, 2, 8], F32, tag="sumsq2")
                        rstd2 = stp.tile([P, 2, 8], F32, tag="rstd2")
                        nmr2 = stp.tile([P, 2, 8], F32, tag="nmr2")
                        pair = []
                    nc.vector.tensor_reduce(
                        sums2[:, t % 2, :],
                        qkraw[:].rearrange("p (g d) -> p g d", d=HD),
                        axis=AX.X,
                        op=ALU.add,
                    )
                    nc.vector.tensor_reduce(
                        sumsq2[:, t % 2, :],
                        sq[:].rearrange("p (g d) -> p g d", d=HD),
                        axis=AX.X,
                        op=ALU.add,
                    )
                    pair.append((t, qkraw))

                    if t % 2 == 1:
                        # var = sumsq/64 - (sums/64)^2; rstd = rsqrt(var+eps)
                        # via 1/var seed + 2 Newton steps (all [128, 16] DVE)
                        sums = sums2[:]
                        v1 = stp.tile([P, 2, 8], F32, tag="v1")
                        nc.vector.tensor_scalar(
                            v1[:], sumsq2[:], 1.0 / HD, EPS, ALU.mult, ALU.add
                        )
                        mm2 = stp.tile([P, 2, 8], F32, tag="mm2")
                        nc.vector.tensor_tensor(mm2[:], sums, sums, ALU.mult)
                        varep = stp.tile([P, 2, 8], F32, tag="varep")
                        nc.vector.scalar_tensor_tensor(
                            varep[:], in0=mm2[:], scalar=-1.0 / (HD * HD),
                            in1=v1[:], op0=ALU.mult, op1=ALU.add,
                        )
                        rcp = stp.tile([P, 2, 8], F32, tag="rcp")
                        nc.vector.reciprocal(rcp[:], varep[:])
                        vh = stp.tile([P, 2, 8], F32, tag="vh")
                        nc.vector.tensor_scalar_mul(vh[:], varep[:], 0.5)
                        yt = stp.tile([P, 2, 8], F32, tag="yt")
                        nc.vector.tensor_scalar(
                            yt[:], rcp[:], 0.433, 0.433, ALU.mult, ALU.add
                        )
                        y = yt[:]
                        t1 = stp.tile([P, 2, 8], F32, tag="t1")
                        t3 = stp.tile([P, 2, 8], F32, tag="t3")
                        for it in range(2):
                            nc.vector.tensor_tensor(t1[:], y, y, ALU.mult)
                            nc.vector.tensor_tensor(t1[:], t1[:], vh[:], ALU.mult)
                            nc.vector.tensor_scalar(
                                t3[:], t1[:], -1.0, 1.5, ALU.mult, ALU.add
                            )
                            dst = rstd2[:] if it == 1 else y
                            nc.vector.tensor_tensor(dst, y, t3[:], ALU.mult)
                        nc.vector.scalar_tensor_tensor(
                            nmr2[:], in0=sums, scalar=-1.0 / HD, in1=rstd2[:],
                            op0=ALU.mult, op1=ALU.mult,
                        )

                        # apply. q needs only the rstd scale: with k fully
                        # normalized, sum_d k_hat = 0 makes q's mean term
                        # cancel exactly in q_hat @ k_hat^T.
                        for tt, qkr in pair:
                            i2 = tt % 2
                            qkl = qklp.tile([P, 2 * GC], BF16, tag="qkl")
                            qv = qkl[:, 0:GC].rearrange("p (g d) -> p g d", d=HD)
                            kv = qkl[:, GC : 2 * GC].rearrange(
                                "p (g d) -> p g d", d=HD
                            )
                            qr = qkr[:, 0:GC].rearrange("p (g d) -> p g d", d=HD)
                            kr = qkr[:, GC : 2 * GC].rearrange(
                                "p (g d) -> p g d", d=HD
                            )
                            nc.vector.tensor_tensor(
                                qv, qr,
                                rstd2[:, i2, 0:4, None].to_broadcast([P, 4, HD]),
                                ALU.mult,
                            )
                            nc.vector.tensor_tensor(
                                kv, kr,
                                rstd2[:, i2, 4:8, None].to_broadcast([P, 4, HD]),
                                ALU.mult,
                            )
                            nc.vector.tensor_tensor(
                                kv, kv,
                                nmr2[:, i2, 4:8, None].to_broadcast([P, 4, HD]),
                                ALU.add,
                            )
                            if ln_affine:
                                nc.vector.tensor_tensor(
                                    qkl[:], qkl[:], srep[:], ALU.mult
                                )
                                nc.vector.tensor_tensor(
                                    qkl[:], qkl[:], lbrep[:], ALU.add
                                )
                            pend2.append((qkl, tt))
                while pend2:
                    emit_transposes(pend2.pop(0))
                    if has_qkv_bias:
                        nc.vector.tensor_tensor(psA, psA, brep[:, 0 : 2 * GC], ALU.add)
                        nc.vector.tensor_tensor(
                            psB, psB, brep[:, 2 * GC : 3 * GC], ALU.add
                        )

                    # LayerNorm stats over head_dim for the 8 q|k segments:
                    # var = sumsq/64 - (sums/64)^2
                    a3 = psA.rearrange("p (g d) -> p g d", d=HD)
                    sq = stp.tile([P, 2 * GC], F32, tag="sq")
                    nc.scalar.square(sq[:], psA)
                    sums = stp.tile([P, 8], F32, tag="sums")
                    nc.vector.tensor_reduce(sums[:], a3, axis=AX.X, op=ALU.add)
                    sumsq = stp.tile([P, 8], F32, tag="sumsq")
                    nc.vector.tensor_reduce(
                        sumsq[:],
                        sq[:].rearrange("p (g d) -> p g d", d=HD),
                        axis=AX.X,
                        op=ALU.add,
                    )
                    v1 = stp.tile([P, 8], F32, tag="v1")
                    nc.vector.tensor_scalar(
                        v1[:], sumsq[:], 1.0 / HD, EPS, ALU.mult, ALU.add
                    )
                    mm2 = stp.tile([P, 8], F32, tag="mm2")
                    nc.vector.tensor_tensor(mm2[:], sums[:], sums[:], ALU.mult)
                    varep = stp.tile([P, 8], F32, tag="varep")
                    nc.vector.scalar_tensor_tensor(
                        varep[:], in0=mm2[:], scalar=-1.0 / (HD * HD), in1=v1[:],
                        op0=ALU.mult, op1=ALU.add,
                    )
                    # rstd = rsqrt(varep) on DVE only (keeps ACT on one table
                    # set): r = 1/varep, seed y0 = 0.433 + 0.433*r (good on
                    # r in [1/3, 3]; LN var of ~N(0,1) data is ~1), then
                    # 3 Newton steps y <- y*(1.5 - 0.5*varep*y^2).
                    rcp = stp.tile([P, 8], F32, tag="rcp")
                    nc.vector.reciprocal(rcp[:], varep[:])
                    vh = stp.tile([P, 8], F32, tag="vh")
                    nc.vector.tensor_scalar_mul(vh[:], varep[:], 0.5)
                    yt = stp.tile([P, 8], F32, tag="yt")
                    nc.vector.tensor_scalar(
                        yt[:], rcp[:], 0.433, 0.433, ALU.mult, ALU.add
                    )
                    y = yt[:]
                    rstd = stp.tile([P, 8], F32, tag="rstd")
                    t1 = stp.tile([P, 8], F32, tag="t1")
                    t3 = stp.tile([P, 8], F32, tag="t3")
                    for it in range(2):
                        nc.vector.tensor_tensor(t1[:], y, y, ALU.mult)
                        nc.vector.tensor_tensor(t1[:], t1[:], vh[:], ALU.mult)
                        nc.vector.tensor_scalar(
                            t3[:], t1[:], -1.0, 1.5, ALU.mult, ALU.add
                        )
                        dst = rstd[:] if it == 1 else y
                        nc.vector.tensor_tensor(dst, y, t3[:], ALU.mult)
                    nmr = stp.tile([P, 8], F32, tag="nmr")
                    nc.vector.scalar_tensor_tensor(
                        nmr[:], in0=sums[:], scalar=-1.0 / HD, in1=rstd[:],
                        op0=ALU.mult, op1=ALU.mult,
                    )

                    qkl = qklp.tile([P, 2 * GC], BF16, tag="qkl")
                    q3 = qkl[:].rearrange("p (g d) -> p g d", d=HD)
                    nc.vector.tensor_tensor(
                        q3, a3, rstd[:, :, None].to_broadcast([P, 8, HD]), ALU.mult
                    )
                    nc.vector.tensor_tensor(
                        q3, q3, nmr[:, :, None].to_broadcast([P, 8, HD]), ALU.add
                    )
                    if ln_affine:
                        nc.vector.tensor_tensor(qkl[:], qkl[:], srep[:], ALU.mult)
                        nc.vector.tensor_tensor(qkl[:], qkl[:], lbrep[:], ALU.add)

                    # v staging (bf16), [tok, head, hd]
                    nc.scalar.copy(
                        vS[:, t, :, 0:HD],
                        psB.rearrange("p (g d) -> p g d", d=HD),
                    )
                    pend = (qkl, t)
                while pend2:
                    emit_transposes(pend2.pop(0))

            # ---- stage B: attention, heads processed in even/odd pairs ----
            # Head pair (2pr, 2pr+1) occupies partition halves 0:64 / 64:128
            # of qkT[:, pr] (q) and qkT[:, 2+pr] (k).  Interleaving the two
            # heads' S matmuls alternates PE row groups so their LDWEIGHTS
            # overlap in-flight matmuls and the matmuls run concurrently.
            def pe_warm(n):
                # dependency-free LDWEIGHTS: keeps the PE-activity monitor
                # warm across known pipeline boundaries where the next real
                # matmul waits on semaphores (an idle window re-throttles the
                # PE clock to 1.2 GHz for several microseconds)
                for _ in range(n):
                    nc.tensor.ldweights(identb[:])

            def emit_proj_ts(kc2, outd, s4p, trange):
                # partial projection for one head-pair chunk of GC
                for t in trange:
                    pp0 = p2.tile([P, TQ], F32, tag="acc", name="pp0")
                    pp1 = p2.tile([P, TQ], F32, tag="acc", name="pp1")
                    nc.tensor.matmul(
                        pp0, oT[:, kc2, ts(t, P)], wp_r[:, kc2, 0:TQ],
                        start=True, stop=True,
                    )
                    nc.tensor.matmul(
                        pp1, oT[:, kc2, ts(t, P)], wp_r[:, kc2, TQ : 2 * TQ],
                        start=True, stop=True,
                    )
                    for n2, pp in ((0, pp0), (1, pp1)):
                        ob = s4p.tile([P, TQ], BF16, tag="ob")
                        nc.vector.tensor_copy(ob[:], pp)
                        nc.sync.dma_start(outd[ts(t, P), ts(n2, TQ)], ob[:])

            with (
                tc.tile_pool(name="p1", bufs=2, space="PSUM") as p1,
                tc.tile_pool(name="s3e", bufs=4) as ep,
                tc.tile_pool(name="s3r", bufs=4) as rp,
                tc.tile_pool(name="s4", bufs=3) as s4p,
            ):
                pe_warm(30)
                for pr in range(2):
                    for s in range(NSLAB):
                        osA = p2.tile([P, TQ], F32, tag="acc", name="osA")
                        osB = p2.tile([P, TQ], F32, tag="acc", name="osB")
                        prev = None
                        k0 = 0
                        for glen in KGROUPS:
                            sA = p1.tile([P, 3, TQ], F32, tag="spt", name="sBa")
                            sB = p1.tile([P, 3, TQ], F32, tag="spt", name="sBb")
                            for j in range(glen):
                                tk = k0 + j
                                nc.tensor.matmul(
                                    sA[:, j],
                                    qkT[0:HD, 2 + pr, ts(tk, P)],
                                    qkT[0:HD, pr, ts(s, TQ)],
                                    start=True,
                                    stop=True,
                                )
                                nc.tensor.matmul(
                                    sB[:, j],
                                    qkT[HD:P, 2 + pr, ts(tk, P)],
                                    qkT[HD:P, pr, ts(s, TQ)],
                                    start=True,
                                    stop=True,
                                )
                            # PV for the previous exp'd group keeps the PE
                            # busy while ACT runs the current exps.
                            if prev is not None:
                                eA, eB, pk0, pglen = prev
                                for j in range(pglen):
                                    tk = pk0 + j
                                    nc.tensor.matmul(
                                        osA[0 : HD + 1, :],
                                        vS[:, tk, 2 * pr, :],
                                        eA[:, j],
                                        start=(tk == 0),
                                        stop=(tk == TCH - 1),
                                    )
                                    nc.tensor.matmul(
                                        osB[0 : HD + 1, :],
                                        vS[:, tk, 2 * pr + 1, :],
                                        eB[:, j],
                                        start=(tk == 0),
                                        stop=(tk == TCH - 1),
                                    )
                            eA = ep.tile([P, 3, TQ], BF16, tag="et")
                            nc.scalar.activation(
                                eA[:, 0:glen], sA[:, 0:glen], ACTF.Exp, scale=SCL
                            )
                            eB = ep.tile([P, 3, TQ], BF16, tag="et")
                            nc.scalar.activation(
                                eB[:, 0:glen], sB[:, 0:glen], ACTF.Exp, scale=SCL
                            )
                            prev = (eA, eB, k0, glen)
                            k0 += glen
                        eA, eB, pk0, pglen = prev
                        for j in range(pglen):
                            tk = pk0 + j
                            nc.tensor.matmul(
                                osA[0 : HD + 1, :],
                                vS[:, tk, 2 * pr, :],
                                eA[:, j],
                                start=(tk == 0),
                                stop=(tk == TCH - 1),
                            )
                            nc.tensor.matmul(
                                osB[0 : HD + 1, :],
                                vS[:, tk, 2 * pr + 1, :],
                                eB[:, j],
                                start=(tk == 0),
                                stop=(tk == TCH - 1),
                            )
                        # evacuate both osums to SBUF right away so the PSUM
                        # slots free for the next unit (a stalled PE gap here
                        # re-throttles the HAM clock); normalize from SBUF.
                        for h2, osum in ((0, osA), (1, osB)):
                            pb = h2 * HD
                            osb = rp.tile([HD + 1, TQ], F32, tag="osb")
                            nc.vector.tensor_copy(osb[:], osum[0 : HD + 1, :])
                            rec = rp.tile([1, TQ], F32, tag="rec")
                            nc.vector.reciprocal(rec[:], osb[HD : HD + 1, :])
                            bcr = rp.tile([HD, TQ], F32, tag="bcr")
                            nc.gpsimd.partition_broadcast(bcr[:], rec[:])
                            nc.vector.tensor_tensor(
                                oT[pb : pb + HD, pr, ts(s, TQ)],
                                osb[0:HD, :],
                                bcr[:],
                                ALU.mult,
                            )
                    # interleave pr=0's projection pass into pr=1's units (2
                    # token-chunks per unit) so the proj MMs fill PE slack
                    # under the exp backlog; pr=1's pass runs at the end.
                    # after the pr=0 head-pair finishes, its projection pass
                    # keeps the PE dense across the pr transition
                    if s == NSLAB - 1 and pr == 0:
                        pe_warm(30)
                        emit_proj_ts(0, out0_d, s4p, range(TCH))



                # pr=1's projection pass
                pe_warm(30)
                emit_proj_ts(1, out1_d, s4p, range(TCH))

    nc.compile()
    return nc


def _get_nc(has_qkv_bias: bool, ln_affine: bool):
    key = (has_qkv_bias, ln_affine)
    if key not in _CACHE:
        _CACHE[key] = _build_nc(*key)
    return _CACHE[key]


def kernel(**inputs) -> np.ndarray:
    global LAST_RESULTS
    from concourse.bass_utils import run_bass_kernel_spmd

    x = np.asarray(inputs["x"], dtype=np.float32)
    qkv_w = np.asarray(inputs["qkv_w"], dtype=np.float32)
    qkv_b = np.asarray(inputs["qkv_b"], dtype=np.float32)
    qn_scale = np.asarray(inputs["qn_scale"], dtype=np.float32)
    qn_bias = np.asarray(inputs["qn_bias"], dtype=np.float32)
    kn_scale = np.asarray(inputs["kn_scale"], dtype=np.float32)
    kn_bias = np.asarray(inputs["kn_bias"], dtype=np.float32)
    proj_w = np.asarray(inputs["proj_w"], dtype=np.float32)
    proj_b = np.asarray(inputs["proj_b"], dtype=np.float32)

    has_qkv_bias = bool(np.any(qkv_b != 0))
    ln_affine = not (
        np.all(qn_scale == 1)
        and np.all(kn_scale == 1)
        and np.all(qn_bias == 0)
        and np.all(kn_bias == 0)
    )
    nc = _get_nc(has_qkv_bias, ln_affine)

    # per-batch transposed bf16 x, laid out [t, p, ko, n] (c = ko*128 + p)
    xT_b = []
    for b in range(B):
        xt = x[b].T.astype(BF16NP)  # [C, NTOK]
        xt = xt.reshape(KC, P, TCH, P).transpose(2, 1, 0, 3)
        xT_b.append(np.ascontiguousarray(xt))

    in_maps = []
    for c in range(8):
        b, g = divmod(c, 4)
        cs = slice(g * GC, (g + 1) * GC)
        wq = np.concatenate(
            [qkv_w[:, cs], qkv_w[:, C:][:, cs], qkv_w[:, 2 * C :][:, cs]], axis=1
        ).astype(BF16NP)
        m = {
            "xT_shard": xT_b[b],
            "wq_shard": np.ascontiguousarray(wq),
            "wp_shard": np.ascontiguousarray(proj_w[cs, :].astype(BF16NP)),
        }
        if has_qkv_bias:
            m["qb_shard"] = np.concatenate(
                [qkv_b[cs], qkv_b[C:][cs], qkv_b[2 * C :][cs]]
            ).reshape(1, 3 * GC)
        if ln_affine:
            m["ln_rows"] = np.stack(
                [
                    np.concatenate([np.tile(qn_scale, G), np.tile(kn_scale, G)]),
                    np.concatenate([np.tile(qn_bias, G), np.tile(kn_bias, G)]),
                ]
            ).astype(np.float32)
        in_maps.append(m)

    res = run_bass_kernel_spmd(
        nc, in_maps, core_ids=list(range(8)), trace=PROFILE
    )
    LAST_RESULTS = res

    out = np.empty((B, NTOK, C), dtype=np.float32)
    for b in range(B):
        acc = res.results[4 * b]["out_part0"].astype(np.float32)
        acc += res.results[4 * b]["out_part1"].astype(np.float32)
        for g in range(1, 4):
            acc += res.results[4 * b + g]["out_part0"].astype(np.float32)
            acc += res.results[4 * b + g]["out_part1"].astype(np.float32)
        out[b] = acc + proj_b[None, :]
    return out
